# revision 22
# baseline (speedup 1.0000x reference)
"""Trainium2 Bass kernel for DICGBase GNN message passing.

Math (per batch element, reference semantics):
  emb  = relu(relu(x W1 + b1) W2 + b2)
  attn = softmax((emb Wa) emb^T)         [rows sum to 1]
  feat = (relu(attn (x Wg1 + bg1)) + relu(x Wn1 + bn1)) / n
  out  = (relu(attn (feat Wg2 + bg2)) + relu(feat Wn2 + bn2)) / n
  returns (out, attn)

Kernel algebra:
  - biases inside an attn-apply ride through (softmax rows sum to 1), so
    bg1/bg2 are added after the apply, where they are per-partition
    (feature-major) or folded via a K=1 ones-row matmul.
  - 1/n scalings are folded into host-prescaled weights:
      F := n*feat = relu(attn(x Wg1) + bg1) + relu(x Wn1 + bn1)
      out = relu(attn(F Wg2') + bg2') + relu(F Wn2' + bn2')
      with Wg2' = Wg2/n^2, bg2' = bg2/n, Wn2' = Wn2/n^2, bn2' = bn2/n.

Sharding: pure data parallel, batch 2048 -> 8 cores x 256 elements.
Per core: 32 chunks x 512 tokens (8 elements of 64 agents each).
Matmuls in bf16 (fp32 accumulate in PSUM); softmax and outputs fp32.
"""

import numpy as np
import ml_dtypes

import concourse.bacc as bacc
import concourse.mybir as mybir
import concourse.tile as tile
from concourse.bass_utils import run_bass_kernel_spmd
from concourse.mybir import AluOpType as ALU, ActivationFunctionType as AF

BS, N_AG, SA, HID = 2048, 64, 128, 256
NCORES = 8
EPC = BS // NCORES            # 256 elements per core
T = EPC * N_AG                # 16384 tokens per core
CHUNK = 512                   # tokens per chunk (8 elements)
import os as _os
NCH = int(_os.environ.get("K_NCH", T // CHUNK))   # 32 chunks normally
STAGE = int(_os.environ.get("K_STAGE", 99))
BF = mybir.dt.bfloat16
F32 = mybir.dt.float32

_CACHE = {}


def _build():
    nc = bacc.Bacc("TRN2", target_bir_lowering=False, debug=False,
                   num_devices=NCORES)

    # ---- DRAM I/O ----
    x_d = nc.dram_tensor("x3", [NCH, SA, CHUNK], BF, kind="ExternalInput")
    w1_d = nc.dram_tensor("w1", [SA, HID], BF, kind="ExternalInput")
    w2_d = nc.dram_tensor("w2", [128, 2, HID], BF, kind="ExternalInput")
    wa_d = nc.dram_tensor("wa", [128, 2, HID], BF, kind="ExternalInput")
    wg1_d = nc.dram_tensor("wg1", [SA, HID], BF, kind="ExternalInput")
    wn1_d = nc.dram_tensor("wn1", [SA, HID], BF, kind="ExternalInput")
    wgn2_d = nc.dram_tensor("wgn2", [128, 2, 512], BF, kind="ExternalInput")
    b2n_d = nc.dram_tensor("b2n", [1, HID], BF, kind="ExternalInput")
    be1_d = nc.dram_tensor("be1", [128, 2], F32, kind="ExternalInput")
    be2_d = nc.dram_tensor("be2", [128, 2], F32, kind="ExternalInput")
    nbg1_d = nc.dram_tensor("nbg1", [128, 2], F32, kind="ExternalInput")
    nbn1_d = nc.dram_tensor("nbn1", [128, 2], F32, kind="ExternalInput")
    bsum_d = nc.dram_tensor("bsum", [128, 2], F32, kind="ExternalInput")
    b2g_d = nc.dram_tensor("b2g", [128, HID], F32, kind="ExternalInput")

    out_d = nc.dram_tensor("out", [NCH * CHUNK, HID], F32,
                           kind="ExternalOutput")
    attn_d = nc.dram_tensor("attn", [NCH * 8, N_AG, N_AG], F32,
                            kind="ExternalOutput")

    # [c, two, p, n, m] view of attn for half-partition stores
    attn_4d = attn_d[:].rearrange("(c p two) n m -> c two p n m", p=4, two=2)

    with tile.TileContext(nc) as tc:
        with tc.tile_pool(name="const", bufs=1) as cp, \
             tc.tile_pool(name="sb", bufs=2) as sb, \
             tc.tile_pool(name="xp", bufs=3) as xp, \
             tc.tile_pool(name="ps", bufs=8, space="PSUM") as pp:

            # ---- persistent weights/constants ----
            w1 = cp.tile([SA, HID], BF, tag="w1")
            nc.sync.dma_start(w1[:], w1_d[:])
            w2 = cp.tile([128, 2, HID], BF, tag="w2")
            nc.sync.dma_start(w2[:], w2_d[:])
            wa = cp.tile([128, 2, HID], BF, tag="wa")
            nc.sync.dma_start(wa[:], wa_d[:])
            wg1 = cp.tile([SA, HID], BF, tag="wg1")
            nc.sync.dma_start(wg1[:], wg1_d[:])
            wn1 = cp.tile([SA, HID], BF, tag="wn1")
            nc.sync.dma_start(wn1[:], wn1_d[:])
            wgn2 = cp.tile([128, 2, 512], BF, tag="wgn2")
            nc.sync.dma_start(wgn2[:], wgn2_d[:])
            b2n = cp.tile([1, HID], BF, tag="b2n")
            nc.sync.dma_start(b2n[:], b2n_d[:])
            be1 = cp.tile([128, 2], F32, tag="be1")
            nc.sync.dma_start(be1[:], be1_d[:])
            be2 = cp.tile([128, 2], F32, tag="be2")
            nc.sync.dma_start(be2[:], be2_d[:])
            nbg1 = cp.tile([128, 2], F32, tag="nbg1")
            nc.sync.dma_start(nbg1[:], nbg1_d[:])
            nbn1 = cp.tile([128, 2], F32, tag="nbn1")
            nc.sync.dma_start(nbn1[:], nbn1_d[:])
            bsum = cp.tile([128, 2], F32, tag="bsum")
            nc.sync.dma_start(bsum[:], bsum_d[:])
            b2g = cp.tile([128, HID], F32, tag="b2g")
            nc.sync.dma_start(b2g[:], b2g_d[:])

            ones1 = cp.tile([1, 128], BF, tag="ones1")
            nc.vector.memset(ones1[:], 1.0)
            onesq = cp.tile([128, 128], BF, tag="onesq")
            nc.vector.memset(onesq[:], 1.0)
            ident = cp.tile([128, 128], BF, tag="ident")
            nc.gpsimd.affine_select(ident[:], onesq[:], pattern=[[1, 128]],
                                    compare_op=ALU.is_equal, fill=0.0,
                                    base=0, channel_multiplier=-1)

            # ---- main loop over chunks ----
            for c in range(NCH):
                X = xp.tile([SA, CHUNK], BF, tag="X")
                nc.sync.dma_start(X[:], x_d[c, :, :])

                # encoder layer 1: H1[i] = relu(W1[:,i]^T X + be1[i])  (fm)
                H1 = []
                for i in range(2):
                    ps = pp.tile([128, 512], F32, tag="ps")
                    nc.tensor.matmul(ps[:], w1[:, i * 128:(i + 1) * 128], X[:],
                                     start=True, stop=True)
                    h = sb.tile([128, CHUNK], BF, tag=f"H1_{i}")
                    nc.scalar.activation(h[:], ps[:], AF.Relu,
                                         bias=be1[:, i:i + 1])
                    H1.append(h)

                if STAGE < 2: continue
                # encoder layer 2: E[i] = relu(sum_k W2[k,i]^T H1[k] + be2[i])
                E = []
                for i in range(2):
                    ps = pp.tile([128, 512], F32, tag="ps")
                    for k in range(2):
                        nc.tensor.matmul(ps[:], w2[:, k, i * 128:(i + 1) * 128],
                                         H1[k][:], start=(k == 0), stop=(k == 1))
                    e = sb.tile([128, CHUNK], BF, tag=f"E_{i}")
                    nc.scalar.activation(e[:], ps[:], AF.Relu,
                                         bias=be2[:, i:i + 1])
                    E.append(e)

                if STAGE < 3: continue
                # q = Wa^T emb (fm, no bias/relu)
                Q = []
                for i in range(2):
                    ps = pp.tile([128, 512], F32, tag="ps")
                    for k in range(2):
                        nc.tensor.matmul(ps[:], wa[:, k, i * 128:(i + 1) * 128],
                                         E[k][:], start=(k == 0), stop=(k == 1))
                    q = sb.tile([128, CHUNK], BF, tag=f"Q_{i}")
                    if i == 0:
                        nc.scalar.copy(q[:], ps[:])
                    else:
                        nc.vector.tensor_copy(q[:], ps[:])
                    Q.append(q)

                # scores, packed [128 = 2 elems x 64 agents, 4 pairs x 64]
                if STAGE < 4: continue
                # NOTE: 8 two-matmul accumulation groups share one PSUM bank;
                # a start=True between another group's k0/k1 would clear its
                # has_written bits, so keep the issue order contiguous.
                sc = pp.tile([128, 512], F32, tag="ps")
                with tc.tile_critical():
                    for e in range(8):
                        p, hf = e // 2, e % 2
                        o = sc[hf * 64:(hf + 1) * 64, p * 64:(p + 1) * 64]
                        for k in range(2):
                            nc.tensor.matmul(o, Q[k][:, e * 64:(e + 1) * 64],
                                             E[k][:, e * 64:(e + 1) * 64],
                                             start=(k == 0), stop=(k == 1),
                                             tile_position=(0, hf * 64))

                if STAGE < 5: continue
                # softmax over free axis within each 64-group (|scores|<~2,
                # so no max subtraction needed); exp accumulates row sums.
                esb = sb.tile([128, 256], F32, tag="esb")
                sums = sb.tile([128, 4], F32, tag="sums")
                for g in range(4):
                    nc.scalar.activation(esb[:, g * 64:(g + 1) * 64],
                                         sc[:, g * 64:(g + 1) * 64], AF.Exp,
                                         accum_out=sums[:, g:g + 1])
                rec = sb.tile([128, 4], F32, tag="rec")
                nc.vector.reciprocal(rec[:], sums[:])
                attn_f = sb.tile([128, 256], F32, tag="attn_f")
                for g in range(4):
                    nc.vector.tensor_scalar_mul(attn_f[:, g * 64:(g + 1) * 64],
                                                esb[:, g * 64:(g + 1) * 64],
                                                rec[:, g:g + 1])
                for hf in range(2):
                    av = attn_4d[c, hf]  # [p, n, m]
                    nc.sync.dma_start(
                        av.rearrange("p n m -> n p m"),
                        attn_f[hf * 64:(hf + 1) * 64, :].rearrange(
                            "n (p m) -> n p m", p=4))
                if STAGE < 6: continue
                attn_b = sb.tile([128, 256], BF, tag="attn_b")
                nc.gpsimd.tensor_copy(attn_b[:], attn_f[:])

                # block-diagonal attn^T per pair: aT[:, p*128:+128] has
                # even elem's attnT at (0:64, 0:64), odd at (64:128, 64:128),
                # zeros off-diagonal (psum memset + diagonal transposes).
                tr = pp.tile([128, 512], F32, tag="ps")
                nc.vector.memset(tr[:], 0.0)
                for e in range(8):
                    p, hf = e // 2, e % 2
                    r = slice(hf * 64, (hf + 1) * 64)
                    nc.tensor.matmul(tr[r, p * 128 + hf * 64:p * 128 + hf * 64 + 64],
                                     attn_b[r, p * 64:(p + 1) * 64],
                                     ident[r, r], start=True, stop=True,
                                     tile_position=(hf * 64, hf * 64))
                aT = sb.tile([128, 512], BF, tag="aT")
                nc.vector.tensor_copy(aT[:], tr[:])

                if STAGE < 7: continue
                # g1 = x Wg1 token-major per 128-token block (pair)
                g1ps = []
                for pp2 in range(2):
                    ps = pp.tile([128, 512], F32, tag="ps")
                    for j in range(2):
                        pr = pp2 * 2 + j
                        nc.tensor.matmul(ps[:, j * 256:(j + 1) * 256],
                                         X[:, pr * 128:(pr + 1) * 128], wg1[:],
                                         start=True, stop=True)
                    g1ps.append(ps)
                g1 = []
                for pr in range(4):
                    g = sb.tile([128, HID], BF, tag=f"g1_{pr}")
                    src = g1ps[pr // 2][:, (pr % 2) * 256:(pr % 2 + 1) * 256]
                    if pr % 2 == 0:
                        nc.scalar.copy(g[:], src)
                    else:
                        nc.vector.tensor_copy(g[:], src)
                    g1.append(g)

                if STAGE < 8: continue
                # n1 = Wn1^T X (fm)
                SUB = int(_os.environ.get("K_SUB", 4))
                n1ps = []
                for i in range(2):
                    ps = pp.tile([128, 512], F32, tag="ps")
                    nc.tensor.matmul(ps[:], wn1[:, i * 128:(i + 1) * 128], X[:],
                                     start=True, stop=True)
                    n1ps.append(ps)
                if SUB == 0:
                    continue

                # apply-1 (fm): feat[i][:, p-pair] = g1[p]^T aTbd[p], K=128
                a1ps = []
                for i in range(2):
                    ps = pp.tile([128, 512], F32, tag="ps")
                    for p in range(4):
                        nc.tensor.matmul(ps[:, p * 128:(p + 1) * 128],
                                         g1[p][:, i * 128:(i + 1) * 128],
                                         aT[:, p * 128:(p + 1) * 128],
                                         start=True, stop=True)
                    a1ps.append(ps)

                # F = relu(A1 + bg1) + relu(N1 + bn1)   (fm, fused two-op DVE)
                SUB = int(_os.environ.get("K_SUB", 4))
                if SUB == 1:
                    continue
                F = []
                for i in range(2):
                    if SUB == 2:
                        f = sb.tile([128, CHUNK], BF, tag=f"F_{i}")
                        nc.vector.tensor_copy(f[:], a1ps[i][:])
                        F.append(f)
                        continue
                    if SUB == 3:
                        r1s = sb.tile([128, CHUNK], BF, tag=f"r1s_{i}")
                        nc.scalar.activation(r1s[:], a1ps[i][:], AF.Relu,
                                             bias=nbg1[:, i:i + 1])
                        r2s = sb.tile([128, CHUNK], BF, tag=f"r2s_{i}")
                        nc.scalar.activation(r2s[:], n1ps[i][:], AF.Relu,
                                             bias=nbn1[:, i:i + 1])
                        f = sb.tile([128, CHUNK], BF, tag=f"F_{i}")
                        nc.vector.tensor_tensor(f[:], r1s[:], r2s[:], ALU.add)
                        F.append(f)
                        continue
                    r2 = sb.tile([128, CHUNK], BF, tag=f"r2_{i}")
                    nc.vector.tensor_scalar(r2[:], n1ps[i][:],
                                            nbn1[:, i:i + 1], bsum[:, i:i + 1],
                                            ALU.max, ALU.add)
                    f = sb.tile([128, CHUNK], BF, tag=f"F_{i}")
                    nc.vector.scalar_tensor_tensor(f[:], a1ps[i][:],
                                                   nbg1[:, i:i + 1], r2[:],
                                                   ALU.max, ALU.add)
                    F.append(f)
                if SUB == 2 or SUB == 3:
                    continue

                if STAGE < 9: continue
                # g2|n2 token-major per block + ones-row bias for n2
                gnps = []
                for pr in range(4):
                    ps = pp.tile([128, 512], F32, tag="ps")
                    for k in range(2):
                        nc.tensor.matmul(ps[:], F[k][:, pr * 128:(pr + 1) * 128],
                                         wgn2[:, k, :], start=(k == 0),
                                         stop=False, skip_group_check=True)
                    nc.tensor.matmul(ps[:, 256:512], ones1[:], b2n[:],
                                     start=False, stop=True,
                                     skip_group_check=True)
                    gnps.append(ps)
                g2 = []
                for pr in range(4):
                    g = sb.tile([128, HID], BF, tag=f"g2_{pr}")
                    nc.vector.scalar_tensor_tensor(g[:], gnps[pr][:, 0:256],
                                                   1.0, b2g[:],
                                                   ALU.mult, ALU.add)
                    g2.append(g)

                if STAGE < 10: continue
                # apply-2 token-major: feat2[pair] in [128, 256]
                f2ps = []
                for pq in range(2):
                    f2t = pp.tile([128, 512], F32, tag="ps", name=f"f2ps{pq}")
                    f2ps.append(f2t)
                for p in range(4):
                    o = f2ps[p // 2][:, (p % 2) * 256:(p % 2 + 1) * 256]
                    nc.tensor.matmul(o, aT[:, p * 128:(p + 1) * 128],
                                     g2[p][:], start=True, stop=True)

                # out = relu(A2) + relu(N2')  (token-major, fp32)
                outt = sb.tile([128, 4, HID], F32, tag="outt")
                for pr in range(4):
                    ra = sb.tile([128, HID], F32, tag=f"ra_{pr % 2}")
                    nc.scalar.activation(
                        ra[:], f2ps[pr // 2][:, (pr % 2) * 256:(pr % 2 + 1) * 256],
                        AF.Relu)
                    nc.vector.scalar_tensor_tensor(outt[:, pr, :],
                                                   gnps[pr][:, 256:512], 0.0,
                                                   ra[:], ALU.max, ALU.add)
                ov = out_d[c * 512:(c + 1) * 512, :].rearrange(
                    "(p t) h -> t p h", p=4)
                nc.sync.dma_start(ov, outt[:])

    nc.compile()
    return nc


def _prep_weights(inputs):
    bf = ml_dtypes.bfloat16
    f32 = np.float32
    g = lambda k: np.asarray(inputs[k], dtype=f32)
    n = float(int(inputs["n_agents"]))
    inv_n = 1.0 / n

    w2pack = lambda w: np.ascontiguousarray(
        w.reshape(2, 128, HID).transpose(1, 0, 2)).astype(bf)

    wg2s = g("gc2_w") * (inv_n * inv_n)
    wn2s = g("nn2_w") * (inv_n * inv_n)
    gn = np.concatenate([wg2s.reshape(2, 128, HID),
                         wn2s.reshape(2, 128, HID)], axis=2)  # [2,128,512]
    wgn2 = np.ascontiguousarray(gn.transpose(1, 0, 2)).astype(bf)

    perpart = lambda b: np.ascontiguousarray(b.reshape(2, 128).T).astype(f32)

    bg1 = g("gc1_b")
    bn1 = g("nn1_b")
    return {
        "w1": g("enc_w1").astype(bf),
        "w2": w2pack(g("enc_w2")),
        "wa": w2pack(g("attn_w")),
        "wg1": g("gc1_w").astype(bf),
        "wn1": g("nn1_w").astype(bf),
        "wgn2": wgn2,
        "b2n": (g("nn2_b") * inv_n).reshape(1, HID).astype(bf),
        "be1": perpart(g("enc_b1")),
        "be2": perpart(g("enc_b2")),
        "nbg1": perpart(-bg1),
        "nbn1": perpart(-bn1),
        "bsum": perpart(bn1 + bg1),
        "b2g": np.tile((g("gc2_b") * inv_n)[None, :], (128, 1)).astype(f32),
    }


def _make_in_maps(x, wmap):
    tk = NCH * CHUNK  # tokens actually processed per core (debug may be < T)
    in_maps = []
    for cid in range(NCORES):
        xc = x[cid * EPC:(cid + 1) * EPC].reshape(T, SA)[:tk]  # [tokens, d]
        x3 = np.ascontiguousarray(
            xc.T.reshape(SA, NCH, CHUNK).transpose(1, 0, 2)
        ).astype(ml_dtypes.bfloat16)
        m = dict(wmap)
        m["x3"] = x3
        in_maps.append(m)
    return in_maps


def kernel(**inputs):
    if "nc" not in _CACHE:
        _CACHE["nc"] = _build()
    nc = _CACHE["nc"]

    x = np.asarray(inputs["x"], dtype=np.float32)
    wmap = _prep_weights(inputs)

    in_maps = _make_in_maps(x, wmap)
    tk = NCH * CHUNK
    res = run_bass_kernel_spmd(nc, in_maps, core_ids=list(range(NCORES)))
    _CACHE["last_result"] = res

    epck = tk // N_AG
    out = np.zeros((BS, N_AG, HID), np.float32)
    attn = np.zeros((BS, N_AG, N_AG), np.float32)
    for cid in range(NCORES):
        out[cid * EPC:cid * EPC + epck] = \
            res.results[cid]["out"].reshape(epck, N_AG, HID)
        attn[cid * EPC:cid * EPC + epck] = res.results[cid]["attn"]
    return out, attn
